# revision 1
# baseline (speedup 1.0000x reference)
"""AutoDiscretizationEmbedding kernel for 8 Trainium2 NeuronCores.

Math per token t (x scalar):  p = x*w1 + b1;  hl = leaky_relu(p, 0.1);
logits = hl + w2 @ hl + b2;  out = softmax(logits) @ emb.

Host-side refactor (f32 numpy on weights only):
  leaky(p) = 0.1*p + 0.9*relu(p);  relu(w1x+b1) = |w1|*max(s*x, s*t) + b1
  with s = sign(w1), t = -b1/w1.  x is replicated to 101 rows with s baked
  in (pure layout), so ONE DVE tensor_scalar (max, mult) yields
  rT = relu - b1.  logits = a*x + c' + W2S @ rT with W2S = 0.9*(I+w2),
  a = 0.1*(I+w2)@w1 riding as contraction row 100 of the single K=101
  matmul (rT row 100 = x), and c' = (I+w2)@b1 + b2 as the ACT exp bias.
  All matmul operands bf16 (fp32 matmuls run ~3x slower on the PE).

Hard-won scheduling facts baked in:
  - DRAM->SBUF loads with <128 partitions serialize on ONE DMA engine
    (~25 GB/s); every loaded tensor is padded to 128 partitions so the
    runtime splits descriptors across all 16 engines.
  - GpSimd compute blocks DVE 2-port perf modes (shared SBUF port pair);
    host-side x replication keeps GpSimd idle so the bf16 DVE op runs
    accelerated (~654ns per [101,1024]).
  - Matmul PSUM output must fit one 2KB bank (N<=512 f32); the softmax
    denominator z needs its own N=1 matmuls (~27ns each in the pipelined
    LDW/MM z-block).
  - Only 4 dma_start loads (vecs, first-pair x, weights, rest-of-x):
    more loads exhaust the ~8-deep HWDGE ring and stall the first store;
    in-loop loads get scheduled behind the store backlog and starve
    late pairs.
  - Stores are chunk-level (1 MiB, 8 KiB contiguous per partition via
    host token permutation); first store pkt ~18us.

Per 1024-token chunk-pair:
  DVE: rT[101,1024] = (xb max tt)*aw;  rc8 = 1/z8;  evictions j odd
  PE:  l_ps[100,1024] = w2a^T @ rT (2x N=512);  8x z8[:,j] = u_j^T @ ones;
       8x o_ps[128,512] = u_j^T @ emb
  ACT: uT[100,1024] = exp(l_ps + c') bf16;  evictions j even (Copy+scale)
  DMA: 2x 1 MiB stores per pair.
Data-parallel over 65536 tokens, 8192 per core (core c == batch item c).
Measured median ~67.2us over 7 HW runs (66.8-67.7 + one 78.3 outlier;
baseline 94.5-102.5us).  Span budget: 6.9us runtime preamble + 11.5us
ramp to first store + 40.2us DMA work (19.1 MB at the ~25 GB/s/engine
ceiling) + 5.8us compute-paced early idle + 8.7us runtime teardown.
"""

import numpy as np

B, S = 8, 8192
BINS, DIM = 100, 512
K1 = BINS + 1
NCORES = 8
NTOK = (B * S) // NCORES
CHUNK = 512
NSUB = CHUNK // 128
NCH = NTOK // CHUNK
NPAIR = NCH // 2
PTOK = 2 * CHUNK

_CACHE = {}


def _build_nc():
    import concourse.tile as tile
    from concourse import bacc, mybir

    f32 = mybir.dt.float32
    bf16 = mybir.dt.bfloat16
    Act = mybir.ActivationFunctionType
    Alu = mybir.AluOpType

    nc = bacc.Bacc("TRN2", target_bir_lowering=False, debug=False,
                   num_devices=NCORES)
    xbs_d = nc.dram_tensor("xbs", [128, NTOK], bf16, kind="ExternalInput").ap()
    wpack_d = nc.dram_tensor("wpack", [128, 616], bf16,
                             kind="ExternalInput").ap()
    vecs_d = nc.dram_tensor("vecs", [128, 4], f32, kind="ExternalInput").ap()
    out_d = nc.dram_tensor("out", [NTOK, DIM], f32, kind="ExternalOutput").ap()

    with tile.TileContext(nc) as tc:
        with (
            tc.tile_pool(name="const", bufs=1) as cpool,
            tc.tile_pool(name="xq", bufs=NPAIR) as xqpool,
            tc.tile_pool(name="rt", bufs=3) as rtpool,
            tc.tile_pool(name="ut", bufs=3) as utpool,
            tc.tile_pool(name="ost", bufs=4) as opool,
            tc.tile_pool(name="rc", bufs=4) as rcpool,
            tc.tile_pool(name="pl", bufs=2, space="PSUM") as pl,
            tc.tile_pool(name="po", bufs=3, space="PSUM") as po,
            tc.tile_pool(name="pz", bufs=1, space="PSUM") as pz,
        ):
            vecs = cpool.tile([128, 4], f32)
            nc.sync.dma_start(vecs[:], vecs_d[:])
            xb0 = xqpool.tile([128, PTOK], bf16)
            nc.sync.dma_start(xb0[:], xbs_d[:, 0:PTOK])
            wpack = cpool.tile([128, 616], bf16)
            nc.sync.dma_start(wpack[:], wpack_d[:])
            xbrest = cpool.tile([128, NTOK - PTOK], bf16)
            nc.sync.dma_start(xbrest[:], xbs_d[:, PTOK:NTOK])

            def xsrc(pr):
                if pr == 0:
                    return xb0[0:K1, :]
                return xbrest[0:K1, (pr - 1) * PTOK:pr * PTOK]

            w2a = wpack[0:K1, 0:BINS]                   # lhsT [101, 100]
            emb_o = wpack[0:BINS, BINS:BINS + DIM]   # [100, 512]
            emb_z = wpack[0:BINS, BINS + DIM:BINS + DIM + 1]  # ones col
            tt_e = vecs[0:K1, 0:1]
            aw_e = vecs[0:K1, 1:2]
            cvec = vecs[0:BINS, 2:3]

            for pr in range(NPAIR):
                t0 = pr * PTOK
                rT = rtpool.tile([K1, PTOK], bf16)
                nc.vector.tensor_scalar(rT[:], xsrc(pr),
                                        tt_e, aw_e,
                                        op0=Alu.max, op1=Alu.mult)

                l_ps = pl.tile([BINS, PTOK], f32)
                nc.tensor.matmul(l_ps[:, 0:CHUNK], w2a, rT[:, 0:CHUNK],
                                 start=True, stop=True)
                nc.tensor.matmul(l_ps[:, CHUNK:PTOK], w2a, rT[:, CHUNK:PTOK],
                                 start=True, stop=True)
                uT = utpool.tile([BINS, PTOK], bf16)
                nc.scalar.activation(uT[:], l_ps[:], Act.Exp, bias=cvec)

                z8 = pz.tile([128, 2 * NSUB], f32)
                for j in range(2 * NSUB):
                    u_j = uT[:, j * 128:(j + 1) * 128]
                    nc.tensor.matmul(z8[:, j:j + 1], u_j, emb_z,
                                     start=True, stop=True)
                rc8 = rcpool.tile([128, 2 * NSUB], f32)
                nc.vector.reciprocal(rc8[:], z8[:])

                ost = opool.tile([128, 2 * NSUB * DIM], f32)
                for j in range(2 * NSUB):
                    u_j = uT[:, j * 128:(j + 1) * 128]
                    o_ps = po.tile([128, DIM], f32)
                    nc.tensor.matmul(o_ps[:], u_j, emb_o, start=True,
                                     stop=True)
                    dst = ost[:, j * DIM:(j + 1) * DIM]
                    if j % 2 == 0:
                        nc.scalar.activation(dst, o_ps[:], Act.Copy,
                                             scale=rc8[:, j:j + 1])
                    else:
                        nc.vector.tensor_scalar_mul(dst, o_ps[:],
                                                    rc8[:, j:j + 1])

                for q in range(2):
                    c0 = t0 + q * CHUNK
                    out_view = out_d[c0:c0 + CHUNK, :].rearrange(
                        "(p a) d -> p a d", p=128)
                    src_v = ost[:, q * NSUB * DIM:(q + 1) * NSUB * DIM]
                    nc.sync.dma_start(
                        out_view, src_v.rearrange("p (a d) -> p a d", d=DIM))
    nc.compile()
    return nc


def _prep_in_maps(x, w1, b1, w2, b2, emb):
    import ml_dtypes
    bf = ml_dtypes.bfloat16

    x = np.ascontiguousarray(np.asarray(x, dtype=np.float32)).reshape(B * S)
    w1 = np.asarray(w1, dtype=np.float32)[:, 0]
    b1 = np.asarray(b1, dtype=np.float32)
    w2 = np.asarray(w2, dtype=np.float32)
    b2 = np.asarray(b2, dtype=np.float32)
    emb = np.asarray(emb, dtype=np.float32)

    w2p = np.eye(BINS, dtype=np.float32) + w2            # I + w2
    a = 0.1 * (w2p @ w1)
    cn = w2p @ b1 + b2                                   # b1-const folded
    w2s = 0.9 * w2p                                      # [j, k]

    tiny = np.abs(w1) < 1e-12
    w1g = np.where(tiny, 1.0, w1)
    s = np.sign(w1g).astype(np.float32)
    t = -b1 / w1g
    aw = np.abs(w1)
    st = s * t
    # rows with w1 == 0: relu(b1) is constant; contribute via cn, zero the row
    if tiny.any():
        cn = cn + 0.9 * w2p[:, tiny] @ np.maximum(b1[tiny], 0.0) \
            - 0.9 * (w2p[:, tiny] @ b1[tiny])
        st[tiny] = 0.0
        aw[tiny] = 0.0
        s[tiny] = 0.0

    wpack = np.zeros((128, 616), dtype=bf)
    wpack[0:BINS, 0:BINS] = w2s.T.astype(bf)             # lhsT rows k<100
    wpack[BINS, 0:BINS] = a.astype(bf)                   # row 100 = a
    wpack[0:BINS, BINS:BINS + DIM] = emb.astype(bf)
    wpack[0:BINS, BINS + DIM] = 1.0

    vecs = np.zeros((128, 4), dtype=np.float32)
    vecs[0:BINS, 0] = st
    vecs[BINS, 0] = -1e30                                # pass x through
    vecs[0:BINS, 1] = aw
    vecs[BINS, 1] = 1.0
    vecs[0:BINS, 2] = cn

    sext = np.concatenate([s, [1.0]]).astype(np.float32)  # [101]

    in_maps = []
    for cid in range(NCORES):
        xc = x[cid * NTOK:(cid + 1) * NTOK]
        # permute: chunk column j*128+p <- chunk token 4p+j so that o_ps
        # partition p of subtile j holds token 4p+j -> contiguous store
        xp = xc.reshape(NCH, 128, NSUB).transpose(0, 2, 1).reshape(NTOK)
        xbs = np.zeros((128, NTOK), dtype=bf)
        xbs[0:K1] = (sext[:, None] * xp[None, :]).astype(bf)
        in_maps.append({"xbs": xbs, "wpack": wpack, "vecs": vecs})
    return in_maps


def _run(in_maps, trace=False, **kw):
    from concourse.bass_utils import run_bass_kernel_spmd
    if "nc" not in _CACHE:
        _CACHE["nc"] = _build_nc()
    return run_bass_kernel_spmd(_CACHE["nc"], in_maps,
                                list(range(NCORES)), trace=trace, **kw)


def kernel(**inputs):
    in_maps = _prep_in_maps(inputs["x"], inputs["w1"], inputs["b1"],
                            inputs["w2"], inputs["b2"], inputs["emb"])
    res = _run(in_maps)
    out = np.stack([res.results[c]["out"] for c in range(NCORES)])
    return out.reshape(B, S, DIM).astype(np.float32, copy=False)



# revision 2
# speedup vs baseline: 1.2548x; 1.2548x over previous
"""AutoDiscretizationEmbedding kernel for 8 Trainium2 NeuronCores.

Math per token t (x scalar):  p = x*w1 + b1;  hl = leaky_relu(p, 0.1);
logits = hl + w2 @ hl + b2;  out = softmax(logits) @ emb.

Host-side refactor (f32 numpy on weights only):
  leaky(p) = 0.1*p + 0.9*relu(p);  relu(w1x+b1) = |w1|*max(s*x, s*t) + b1
  with s = sign(w1), t = -b1/w1.  x is replicated to 101 rows with s baked
  in (pure layout), so ONE DVE tensor_scalar (max, mult) yields
  rT = relu - b1.  logits = a*x + c' + W2S @ rT with W2S = 0.9*(I+w2),
  a = 0.1*(I+w2)@w1 riding as contraction row 100 of the single K=101
  matmul (rT row 100 = x), and c' = (I+w2)@b1 + b2 as the ACT exp bias.
  All matmul operands bf16; the OUTPUT is stored bf16 (host upcasts to
  f32) which halves store traffic; rel err ~4.8e-3 vs the 2e-2 gate.

Schedule (per 1024-token pair; explicitly software-pipelined emission so
the static Tile schedule overlaps pairs):
  DVE:  rT[101,1024] = (xb max tt)*aw;  4x quarter reciprocal [128,2];
        4 of 8 evictions (tensor_scalar_mul by 1/z)
  PE :  l_ps[100,1024] = w2a^T @ rT (2x N=512);  per subtile j:
        o_ps[128,512] = u_j^T @ emb then z[:,j] = u_j^T @ ones (shares
        the loaded weights; both LDWs shadow under 512-col matmuls)
  ACT:  uT[100,1024] = exp(l_ps + c') bf16;  4 of 8 evictions (Copy*1/z)
  DMA:  2x 512KB bf16 stores per pair (last pair's chunk 1 in halves)
  Pair p+1's rT/l/exp are emitted at j==0/1/2 inside pair p's o-block so
  the PE never waits on exp at pair boundaries (was ~1.2us/pair).

Hard-won scheduling facts baked in:
  - DRAM->SBUF loads with <128 partitions serialize on ONE DMA engine
    (~25 GB/s); every loaded tensor is padded to 128 partitions.
  - PE runs at the 1.2 GHz MID p-state (426ns per 512-col bf16 matmul);
    the 2.4 GHz p-state never engages even in 14us zero-gap stretches
    (some whole runs draw a boosted PE clock; then ACT/DVE pace).
  - GpSimd tensor_scalar is ~27x slower than DVE (software Q7 path) -
    never put the rT op there despite the idle engine.
  - Eviction (PSUM f32 -> SBUF bf16, scale by 1/z) cannot merge across
    subtiles: the per-token 1/z scale is per-partition only within one
    512-col subtile.  DVE 2x modes need 16-bit inputs, PSUM is f32.
  - The Tile static scheduler is extremely sensitive to emission order;
    load splits / extra dma_starts can cost ~9us.  This emission order
    (w2a early, xb0a before wpack_a, interleaved l0/exp0a/l1/exp0b
    prologue, injection points j==0/1/2) is a tuned local optimum.
  - PSUM banks: pl 1x[100,1024] (2) + po 5x[128,512] (5) + z16 (1) = 8.
  - Loads: vecs, xb0a, w2a cols, xb0b, emb cols, pair-1 x, rest-of-x;
    the first compute is gated only on vecs+xb0a+w2a (~160KB).

Measured over 7 HW runs: median 55.6us, min 52.2us (baseline 72.5us;
occasional PE-boosted draws land 52-53us).  Span: ~8.7us runtime
preamble + ~1.6us load/ramp + ~39us PE-paced window (PE busy ~37.3us,
gaps ~1.5us) + ~3.3us store tail + ~2.7us teardown.
"""

import numpy as np

B, S = 8, 8192
BINS, DIM = 100, 512
K1 = BINS + 1
NCORES = 8
NTOK = (B * S) // NCORES
CHUNK = 512
NSUB = CHUNK // 128
NCH = NTOK // CHUNK
NPAIR = NCH // 2
PTOK = 2 * CHUNK

_CACHE = {}


def _build_nc():
    import concourse.tile as tile
    from concourse import bacc, mybir

    f32 = mybir.dt.float32
    bf16 = mybir.dt.bfloat16
    Act = mybir.ActivationFunctionType
    Alu = mybir.AluOpType

    nc = bacc.Bacc("TRN2", target_bir_lowering=False, debug=False,
                   num_devices=NCORES)
    xbs_d = nc.dram_tensor("xbs", [128, NTOK], bf16, kind="ExternalInput").ap()
    wpack_d = nc.dram_tensor("wpack", [128, 616], bf16,
                             kind="ExternalInput").ap()
    vecs_d = nc.dram_tensor("vecs", [128, 4], f32, kind="ExternalInput").ap()
    out_d = nc.dram_tensor("out", [NTOK, DIM], bf16, kind="ExternalOutput").ap()

    with tile.TileContext(nc) as tc:
        with (
            tc.tile_pool(name="const", bufs=1) as cpool,
            tc.tile_pool(name="xq", bufs=1) as xqpool,
            tc.tile_pool(name="rt", bufs=3) as rtpool,
            tc.tile_pool(name="ut", bufs=3) as utpool,
            tc.tile_pool(name="ost", bufs=4) as opool,
            tc.tile_pool(name="rc", bufs=4) as rcpool,
            tc.tile_pool(name="pl", bufs=1, space="PSUM") as pl,
            tc.tile_pool(name="po", bufs=5, space="PSUM") as po,
            tc.tile_pool(name="pz", bufs=1, space="PSUM") as pz,
        ):
            vecs = cpool.tile([128, 4], f32)
            nc.sync.dma_start(vecs[:], vecs_d[:])
            xb0 = xqpool.tile([128, PTOK], bf16)
            nc.sync.dma_start(xb0[:, 0:CHUNK], xbs_d[:, 0:CHUNK])
            wpack = cpool.tile([128, 616], bf16)
            nc.sync.dma_start(wpack[:, 0:BINS], wpack_d[:, 0:BINS])
            nc.sync.dma_start(xb0[:, CHUNK:PTOK], xbs_d[:, CHUNK:PTOK])
            nc.sync.dma_start(wpack[:, BINS:616], wpack_d[:, BINS:616])
            xbrest = cpool.tile([128, NTOK - PTOK], bf16)
            nc.sync.dma_start(xbrest[:, 0:PTOK], xbs_d[:, PTOK:2 * PTOK])
            nc.sync.dma_start(xbrest[:, PTOK:], xbs_d[:, 2 * PTOK:NTOK])

            def xsrc(pr):
                if pr == 0:
                    return xb0[0:K1, :]
                return xbrest[0:K1, (pr - 1) * PTOK:pr * PTOK]

            w2a = wpack[0:K1, 0:BINS]                   # lhsT [101, 100]
            emb_o = wpack[0:BINS, BINS:BINS + DIM]   # [100, 512]
            emb_z = wpack[0:BINS, BINS + DIM:BINS + DIM + 1]  # ones col
            tt_e = vecs[0:K1, 0:1]
            aw_e = vecs[0:K1, 1:2]
            cvec = vecs[0:BINS, 2:3]

            def emit_rt(pr):
                rT = rtpool.tile([K1, PTOK], bf16)
                if pr == 0:
                    nc.vector.tensor_scalar(rT[:, 0:CHUNK],
                                            xsrc(0)[:, 0:CHUNK], tt_e, aw_e,
                                            op0=Alu.max, op1=Alu.mult)
                    nc.vector.tensor_scalar(rT[:, CHUNK:PTOK],
                                            xsrc(0)[:, CHUNK:PTOK], tt_e, aw_e,
                                            op0=Alu.max, op1=Alu.mult)
                else:
                    nc.vector.tensor_scalar(rT[:], xsrc(pr), tt_e, aw_e,
                                            op0=Alu.max, op1=Alu.mult)
                return rT

            def emit_l(rT):
                l_ps = pl.tile([BINS, PTOK], f32)
                nc.tensor.matmul(l_ps[:, 0:CHUNK], w2a, rT[:, 0:CHUNK],
                                 start=True, stop=True)
                nc.tensor.matmul(l_ps[:, CHUNK:PTOK], w2a, rT[:, CHUNK:PTOK],
                                 start=True, stop=True)
                return l_ps

            def emit_exp(l_ps, split):
                uT = utpool.tile([BINS, PTOK], bf16)
                if split:
                    nc.scalar.activation(uT[:, 0:CHUNK], l_ps[:, 0:CHUNK],
                                         Act.Exp, bias=cvec)
                    nc.scalar.activation(uT[:, CHUNK:PTOK], l_ps[:, CHUNK:PTOK],
                                         Act.Exp, bias=cvec)
                else:
                    nc.scalar.activation(uT[:], l_ps[:], Act.Exp, bias=cvec)
                return uT

            # prologue: pair 0 front of pipeline, interleaved so exp0a
            # runs right after l0 instead of waiting for l1 as well
            rT = emit_rt(0)
            l_ps = pl.tile([BINS, PTOK], f32)
            uT = utpool.tile([BINS, PTOK], bf16)
            nc.tensor.matmul(l_ps[:, 0:CHUNK], w2a, rT[:, 0:CHUNK],
                             start=True, stop=True)
            nc.scalar.activation(uT[:, 0:CHUNK], l_ps[:, 0:CHUNK],
                                 Act.Exp, bias=cvec)
            nc.tensor.matmul(l_ps[:, CHUNK:PTOK], w2a, rT[:, CHUNK:PTOK],
                             start=True, stop=True)
            nc.scalar.activation(uT[:, CHUNK:PTOK], l_ps[:, CHUNK:PTOK],
                                 Act.Exp, bias=cvec)

            z16 = pz.tile([128, 4 * NSUB], f32)

            for pr in range(NPAIR):
                t0 = pr * PTOK
                zb = (pr % 2) * 2 * NSUB
                z8 = z16[:, zb:zb + 2 * NSUB]
                rc8 = rcpool.tile([128, 2 * NSUB], f32)
                ost = opool.tile([128, 2 * NSUB * DIM], bf16)
                o_tiles = []
                for j in range(2 * NSUB):
                    u_j = uT[:, j * 128:(j + 1) * 128]
                    o_ps = po.tile([128, DIM], f32)
                    nc.tensor.matmul(o_ps[:], u_j, emb_o, start=True,
                                     stop=True)
                    nc.tensor.matmul(z8[:, j:j + 1], u_j, emb_z,
                                     start=True, stop=True)
                    o_tiles.append(o_ps)
                    if pr + 1 < NPAIR:
                        if j == 0:
                            rT_n = emit_rt(pr + 1)
                        elif j == 1:
                            l_n = emit_l(rT_n)
                        elif j == 2:
                            uT_n = emit_exp(l_n, split=False)
                    if j % 2 == 1:
                        q = j // 2
                        nc.vector.reciprocal(rc8[:, 2 * q:2 * q + 2],
                                             z8[:, 2 * q:2 * q + 2])
                        act_set = (0, 2, 4, 6)
                        _evict(nc, Act, ost, o_tiles[j - 1], rc8, j - 1,
                               (j - 1) in act_set)
                        _evict(nc, Act, ost, o_tiles[j], rc8, j,
                               j in act_set)
                    if j == NSUB - 1:
                        _store(nc, out_d, ost, t0, 0)
                    if pr == NPAIR - 1 and j == NSUB + 1:
                        _store_half(nc, out_d, ost, t0, 1, 0)
                if pr == NPAIR - 1:
                    _store_half(nc, out_d, ost, t0, 1, 1)
                else:
                    _store(nc, out_d, ost, t0, 1)
                if pr + 1 < NPAIR:
                    uT = uT_n
    nc.compile()
    return nc


def _evict(nc, Act, ost, o_ps, rc8, j, use_act):
    dst = ost[:, j * DIM:(j + 1) * DIM]
    if use_act:
        nc.scalar.activation(dst, o_ps[:], Act.Copy, scale=rc8[:, j:j + 1])
    else:
        nc.vector.tensor_scalar_mul(dst, o_ps[:], rc8[:, j:j + 1])


def _store(nc, out_d, ost, t0, q):
    c0 = t0 + q * CHUNK
    out_view = out_d[c0:c0 + CHUNK, :].rearrange("(p a) d -> p a d", p=128)
    src_v = ost[:, q * NSUB * DIM:(q + 1) * NSUB * DIM]
    nc.sync.dma_start(out_view, src_v.rearrange("p (a d) -> p a d", d=DIM))


def _store_half(nc, out_d, ost, t0, q, h):
    c0 = t0 + q * CHUNK
    out_view = out_d[c0:c0 + CHUNK, :].rearrange(
        "(p a) d -> p a d", p=128)[:, 2 * h:2 * h + 2, :]
    lo = (q * NSUB + 2 * h) * DIM
    src_v = ost[:, lo:lo + 2 * DIM]
    nc.sync.dma_start(out_view, src_v.rearrange("p (a d) -> p a d", d=DIM))


def _prep_in_maps(x, w1, b1, w2, b2, emb):
    import ml_dtypes
    bf = ml_dtypes.bfloat16

    x = np.ascontiguousarray(np.asarray(x, dtype=np.float32)).reshape(B * S)
    w1 = np.asarray(w1, dtype=np.float32)[:, 0]
    b1 = np.asarray(b1, dtype=np.float32)
    w2 = np.asarray(w2, dtype=np.float32)
    b2 = np.asarray(b2, dtype=np.float32)
    emb = np.asarray(emb, dtype=np.float32)

    w2p = np.eye(BINS, dtype=np.float32) + w2            # I + w2
    a = 0.1 * (w2p @ w1)
    cn = w2p @ b1 + b2                                   # b1-const folded
    w2s = 0.9 * w2p                                      # [j, k]

    tiny = np.abs(w1) < 1e-12
    w1g = np.where(tiny, 1.0, w1)
    s = np.sign(w1g).astype(np.float32)
    t = -b1 / w1g
    aw = np.abs(w1)
    st = s * t
    # rows with w1 == 0: relu(b1) is constant; contribute via cn, zero the row
    if tiny.any():
        cn = cn + 0.9 * w2p[:, tiny] @ np.maximum(b1[tiny], 0.0) \
            - 0.9 * (w2p[:, tiny] @ b1[tiny])
        st[tiny] = 0.0
        aw[tiny] = 0.0
        s[tiny] = 0.0

    wpack = np.zeros((128, 616), dtype=bf)
    wpack[0:BINS, 0:BINS] = w2s.T.astype(bf)             # lhsT rows k<100
    wpack[BINS, 0:BINS] = a.astype(bf)                   # row 100 = a
    wpack[0:BINS, BINS:BINS + DIM] = emb.astype(bf)
    wpack[0:BINS, BINS + DIM] = 1.0

    vecs = np.zeros((128, 4), dtype=np.float32)
    vecs[0:BINS, 0] = st
    vecs[BINS, 0] = -1e30                                # pass x through
    vecs[0:BINS, 1] = aw
    vecs[BINS, 1] = 1.0
    vecs[0:BINS, 2] = cn

    sext = np.concatenate([s, [1.0]]).astype(np.float32)  # [101]

    in_maps = []
    for cid in range(NCORES):
        xc = x[cid * NTOK:(cid + 1) * NTOK]
        # permute: chunk column j*128+p <- chunk token 4p+j so that o_ps
        # partition p of subtile j holds token 4p+j -> contiguous store
        xp = xc.reshape(NCH, 128, NSUB).transpose(0, 2, 1).reshape(NTOK)
        xbs = np.zeros((128, NTOK), dtype=bf)
        xbs[0:K1] = (sext[:, None] * xp[None, :]).astype(bf)
        in_maps.append({"xbs": xbs, "wpack": wpack, "vecs": vecs})
    return in_maps


def _run(in_maps, trace=False, **kw):
    from concourse.bass_utils import run_bass_kernel_spmd
    if "nc" not in _CACHE:
        _CACHE["nc"] = _build_nc()
    return run_bass_kernel_spmd(_CACHE["nc"], in_maps,
                                list(range(NCORES)), trace=trace, **kw)


def kernel(**inputs):
    in_maps = _prep_in_maps(inputs["x"], inputs["w1"], inputs["b1"],
                            inputs["w2"], inputs["b2"], inputs["emb"])
    res = _run(in_maps)
    out = np.stack([res.results[c]["out"] for c in range(NCORES)])
    return out.reshape(B, S, DIM).astype(np.float32, copy=False)


# revision 3
# speedup vs baseline: 1.3556x; 1.0803x over previous
"""AutoDiscretizationEmbedding kernel for 8 Trainium2 NeuronCores.

Math per token t (x scalar):  p = x*w1 + b1;  hl = leaky_relu(p, 0.1);
logits = hl + w2 @ hl + b2;  out = softmax(logits) @ emb.

Host-side refactor (f32 numpy on weights only):
  leaky(p) = 0.1*p + 0.9*relu(p);  relu(w1x+b1) = |w1|*max(s*x, s*t) + b1
  with s = sign(w1), t = -b1/w1.  x is replicated to 101 rows with s baked
  in (pure layout), so ONE DVE tensor_scalar (max, mult) yields
  rT = relu - b1.  logits = a*x + c' + W2S @ rT with W2S = 0.9*(I+w2),
  a = 0.1*(I+w2)@w1 riding as contraction row 100 of the single K=101
  matmul (rT row 100 = x), and c' = (I+w2)@b1 + b2 as the ACT exp bias.
  All matmul operands bf16; the OUTPUT is stored bf16 (host upcasts to
  f32) which halves store traffic; rel err ~4.8e-3 vs the 2e-2 gate.

Schedule (per 1024-token pair; explicitly software-pipelined emission so
the static Tile schedule overlaps pairs):
  DVE:  rT[101,1024] = (xb max tt)*aw;  4x quarter reciprocal [128,2];
        4 of 8 evictions (tensor_scalar_mul by 1/z)
  PE :  l_ps[100,1024] = w2a^T @ rT (2x N=512);  per subtile j:
        o_ps[128,512] = u_j^T @ emb then z[:,j] = u_j^T @ ones (shares
        the loaded weights; both LDWs shadow under 512-col matmuls)
  ACT:  uT[100,1024] = exp(l_ps + c') bf16;  4 of 8 evictions (Copy*1/z)
  DMA:  2x 512KB bf16 stores per pair (last pair's chunk 1 in halves)
  Pair p+1's rT/l/exp are emitted at j==0/1/2 inside pair p's o-block so
  the PE never waits on exp at pair boundaries (was ~1.2us/pair).

Hard-won scheduling facts baked in:
  - DRAM->SBUF loads with <128 partitions serialize on ONE DMA engine
    (~25 GB/s); every loaded tensor is padded to 128 partitions.
  - PE runs at the 1.2 GHz MID p-state (426ns per 512-col bf16 matmul);
    the 2.4 GHz p-state never engages even in 14us zero-gap stretches
    (some whole runs draw a boosted PE clock; then ACT/DVE pace).
  - GpSimd tensor_scalar is ~27x slower than DVE (software Q7 path) -
    never put the rT op there despite the idle engine.
  - Eviction (PSUM f32 -> SBUF bf16, scale by 1/z) cannot merge across
    subtiles: the per-token 1/z scale is per-partition only within one
    512-col subtile.  DVE 2x modes need 16-bit inputs, PSUM is f32.
  - The Tile static scheduler is extremely sensitive to emission order;
    load splits / extra dma_starts can cost ~9us.  This emission order
    (w2a early, xb0a before wpack_a, interleaved l0/exp0a/l1/exp0b
    prologue, injection points j==0/1/2) is a tuned local optimum.
  - PSUM banks: pl 1x[100,1024] (2) + po 5x[128,512] (5) + z16 (1) = 8.
  - Loads: vecs, xb0a, w2a cols, xb0b, emb cols, pair-1 x, rest-of-x;
    the first compute is gated only on vecs+xb0a+w2a (~160KB).

Measured: median ~55.2us, min 51.6us over 10 HW runs (baseline 72.5us;
occasional PE-boosted draws land 51-53us).  utpool bufs=4 (not 3) gives
exp an extra uT buffer of headroom against the o/z weight-load readers.  Span: ~8.7us runtime
preamble + ~1.6us load/ramp + ~39us PE-paced window (PE busy ~37.3us,
gaps ~1.5us) + ~3.3us store tail + ~2.7us teardown.
"""

import numpy as np

B, S = 8, 8192
BINS, DIM = 100, 512
K1 = BINS + 1
NCORES = 8
NTOK = (B * S) // NCORES
CHUNK = 512
NSUB = CHUNK // 128
NCH = NTOK // CHUNK
NPAIR = NCH // 2
PTOK = 2 * CHUNK

_CACHE = {}


def _build_nc():
    import concourse.tile as tile
    from concourse import bacc, mybir

    f32 = mybir.dt.float32
    bf16 = mybir.dt.bfloat16
    Act = mybir.ActivationFunctionType
    Alu = mybir.AluOpType

    nc = bacc.Bacc("TRN2", target_bir_lowering=False, debug=False,
                   num_devices=NCORES)
    xbs_d = nc.dram_tensor("xbs", [128, NTOK], bf16, kind="ExternalInput").ap()
    wpack_d = nc.dram_tensor("wpack", [128, 616], bf16,
                             kind="ExternalInput").ap()
    vecs_d = nc.dram_tensor("vecs", [128, 4], f32, kind="ExternalInput").ap()
    out_d = nc.dram_tensor("out", [NTOK, DIM], bf16, kind="ExternalOutput").ap()

    with tile.TileContext(nc) as tc:
        with (
            tc.tile_pool(name="const", bufs=1) as cpool,
            tc.tile_pool(name="xq", bufs=1) as xqpool,
            tc.tile_pool(name="rt", bufs=3) as rtpool,
            tc.tile_pool(name="ut", bufs=4) as utpool,
            tc.tile_pool(name="ost", bufs=4) as opool,
            tc.tile_pool(name="rc", bufs=4) as rcpool,
            tc.tile_pool(name="pl", bufs=1, space="PSUM") as pl,
            tc.tile_pool(name="po", bufs=5, space="PSUM") as po,
            tc.tile_pool(name="pz", bufs=1, space="PSUM") as pz,
        ):
            vecs = cpool.tile([128, 4], f32)
            nc.sync.dma_start(vecs[:], vecs_d[:])
            xb0 = xqpool.tile([128, PTOK], bf16)
            nc.sync.dma_start(xb0[:, 0:CHUNK], xbs_d[:, 0:CHUNK])
            wpack = cpool.tile([128, 616], bf16)
            nc.sync.dma_start(wpack[:, 0:BINS], wpack_d[:, 0:BINS])
            nc.sync.dma_start(xb0[:, CHUNK:PTOK], xbs_d[:, CHUNK:PTOK])
            nc.sync.dma_start(wpack[:, BINS:616], wpack_d[:, BINS:616])
            xbrest = cpool.tile([128, NTOK - PTOK], bf16)
            nc.sync.dma_start(xbrest[:, 0:PTOK], xbs_d[:, PTOK:2 * PTOK])
            nc.sync.dma_start(xbrest[:, PTOK:], xbs_d[:, 2 * PTOK:NTOK])

            def xsrc(pr):
                if pr == 0:
                    return xb0[0:K1, :]
                return xbrest[0:K1, (pr - 1) * PTOK:pr * PTOK]

            w2a = wpack[0:K1, 0:BINS]                   # lhsT [101, 100]
            emb_o = wpack[0:BINS, BINS:BINS + DIM]   # [100, 512]
            emb_z = wpack[0:BINS, BINS + DIM:BINS + DIM + 1]  # ones col
            tt_e = vecs[0:K1, 0:1]
            aw_e = vecs[0:K1, 1:2]
            cvec = vecs[0:BINS, 2:3]

            def emit_rt(pr):
                rT = rtpool.tile([K1, PTOK], bf16)
                if pr == 0:
                    nc.vector.tensor_scalar(rT[:, 0:CHUNK],
                                            xsrc(0)[:, 0:CHUNK], tt_e, aw_e,
                                            op0=Alu.max, op1=Alu.mult)
                    nc.vector.tensor_scalar(rT[:, CHUNK:PTOK],
                                            xsrc(0)[:, CHUNK:PTOK], tt_e, aw_e,
                                            op0=Alu.max, op1=Alu.mult)
                else:
                    nc.vector.tensor_scalar(rT[:], xsrc(pr), tt_e, aw_e,
                                            op0=Alu.max, op1=Alu.mult)
                return rT

            def emit_l(rT):
                l_ps = pl.tile([BINS, PTOK], f32)
                nc.tensor.matmul(l_ps[:, 0:CHUNK], w2a, rT[:, 0:CHUNK],
                                 start=True, stop=True)
                nc.tensor.matmul(l_ps[:, CHUNK:PTOK], w2a, rT[:, CHUNK:PTOK],
                                 start=True, stop=True)
                return l_ps

            def emit_exp(l_ps, split):
                uT = utpool.tile([BINS, PTOK], bf16)
                if split:
                    nc.scalar.activation(uT[:, 0:CHUNK], l_ps[:, 0:CHUNK],
                                         Act.Exp, bias=cvec)
                    nc.scalar.activation(uT[:, CHUNK:PTOK], l_ps[:, CHUNK:PTOK],
                                         Act.Exp, bias=cvec)
                else:
                    nc.scalar.activation(uT[:], l_ps[:], Act.Exp, bias=cvec)
                return uT

            # prologue: pair 0 front of pipeline, interleaved so exp0a
            # runs right after l0 instead of waiting for l1 as well
            rT = emit_rt(0)
            l_ps = pl.tile([BINS, PTOK], f32)
            uT = utpool.tile([BINS, PTOK], bf16)
            nc.tensor.matmul(l_ps[:, 0:CHUNK], w2a, rT[:, 0:CHUNK],
                             start=True, stop=True)
            nc.scalar.activation(uT[:, 0:CHUNK], l_ps[:, 0:CHUNK],
                                 Act.Exp, bias=cvec)
            nc.tensor.matmul(l_ps[:, CHUNK:PTOK], w2a, rT[:, CHUNK:PTOK],
                             start=True, stop=True)
            nc.scalar.activation(uT[:, CHUNK:PTOK], l_ps[:, CHUNK:PTOK],
                                 Act.Exp, bias=cvec)

            z16 = pz.tile([128, 4 * NSUB], f32)

            for pr in range(NPAIR):
                t0 = pr * PTOK
                zb = (pr % 2) * 2 * NSUB
                z8 = z16[:, zb:zb + 2 * NSUB]
                rc8 = rcpool.tile([128, 2 * NSUB], f32)
                ost = opool.tile([128, 2 * NSUB * DIM], bf16)
                o_tiles = []
                for j in range(2 * NSUB):
                    u_j = uT[:, j * 128:(j + 1) * 128]
                    o_ps = po.tile([128, DIM], f32)
                    nc.tensor.matmul(o_ps[:], u_j, emb_o, start=True,
                                     stop=True)
                    nc.tensor.matmul(z8[:, j:j + 1], u_j, emb_z,
                                     start=True, stop=True)
                    o_tiles.append(o_ps)
                    if pr + 1 < NPAIR:
                        if j == 0:
                            rT_n = emit_rt(pr + 1)
                        elif j == 1:
                            l_n = emit_l(rT_n)
                        elif j == 2:
                            uT_n = emit_exp(l_n, split=False)
                    if j % 2 == 1:
                        q = j // 2
                        nc.vector.reciprocal(rc8[:, 2 * q:2 * q + 2],
                                             z8[:, 2 * q:2 * q + 2])
                        act_set = (0, 2, 4, 6)
                        _evict(nc, Act, ost, o_tiles[j - 1], rc8, j - 1,
                               (j - 1) in act_set)
                        _evict(nc, Act, ost, o_tiles[j], rc8, j,
                               j in act_set)
                    if j == NSUB - 1:
                        _store(nc, out_d, ost, t0, 0)
                    if pr == NPAIR - 1 and j == NSUB + 1:
                        _store_half(nc, out_d, ost, t0, 1, 0)
                if pr == NPAIR - 1:
                    _store_half(nc, out_d, ost, t0, 1, 1)
                else:
                    _store(nc, out_d, ost, t0, 1)
                if pr + 1 < NPAIR:
                    uT = uT_n
    nc.compile()
    return nc


def _evict(nc, Act, ost, o_ps, rc8, j, use_act):
    dst = ost[:, j * DIM:(j + 1) * DIM]
    if use_act:
        nc.scalar.activation(dst, o_ps[:], Act.Copy, scale=rc8[:, j:j + 1])
    else:
        nc.vector.tensor_scalar_mul(dst, o_ps[:], rc8[:, j:j + 1])


def _store(nc, out_d, ost, t0, q):
    c0 = t0 + q * CHUNK
    out_view = out_d[c0:c0 + CHUNK, :].rearrange("(p a) d -> p a d", p=128)
    src_v = ost[:, q * NSUB * DIM:(q + 1) * NSUB * DIM]
    nc.sync.dma_start(out_view, src_v.rearrange("p (a d) -> p a d", d=DIM))


def _store_half(nc, out_d, ost, t0, q, h):
    c0 = t0 + q * CHUNK
    out_view = out_d[c0:c0 + CHUNK, :].rearrange(
        "(p a) d -> p a d", p=128)[:, 2 * h:2 * h + 2, :]
    lo = (q * NSUB + 2 * h) * DIM
    src_v = ost[:, lo:lo + 2 * DIM]
    nc.sync.dma_start(out_view, src_v.rearrange("p (a d) -> p a d", d=DIM))


def _prep_in_maps(x, w1, b1, w2, b2, emb):
    import ml_dtypes
    bf = ml_dtypes.bfloat16

    x = np.ascontiguousarray(np.asarray(x, dtype=np.float32)).reshape(B * S)
    w1 = np.asarray(w1, dtype=np.float32)[:, 0]
    b1 = np.asarray(b1, dtype=np.float32)
    w2 = np.asarray(w2, dtype=np.float32)
    b2 = np.asarray(b2, dtype=np.float32)
    emb = np.asarray(emb, dtype=np.float32)

    w2p = np.eye(BINS, dtype=np.float32) + w2            # I + w2
    a = 0.1 * (w2p @ w1)
    cn = w2p @ b1 + b2                                   # b1-const folded
    w2s = 0.9 * w2p                                      # [j, k]

    tiny = np.abs(w1) < 1e-12
    w1g = np.where(tiny, 1.0, w1)
    s = np.sign(w1g).astype(np.float32)
    t = -b1 / w1g
    aw = np.abs(w1)
    st = s * t
    # rows with w1 == 0: relu(b1) is constant; contribute via cn, zero the row
    if tiny.any():
        cn = cn + 0.9 * w2p[:, tiny] @ np.maximum(b1[tiny], 0.0) \
            - 0.9 * (w2p[:, tiny] @ b1[tiny])
        st[tiny] = 0.0
        aw[tiny] = 0.0
        s[tiny] = 0.0

    wpack = np.zeros((128, 616), dtype=bf)
    wpack[0:BINS, 0:BINS] = w2s.T.astype(bf)             # lhsT rows k<100
    wpack[BINS, 0:BINS] = a.astype(bf)                   # row 100 = a
    wpack[0:BINS, BINS:BINS + DIM] = emb.astype(bf)
    wpack[0:BINS, BINS + DIM] = 1.0

    vecs = np.zeros((128, 4), dtype=np.float32)
    vecs[0:BINS, 0] = st
    vecs[BINS, 0] = -1e30                                # pass x through
    vecs[0:BINS, 1] = aw
    vecs[BINS, 1] = 1.0
    vecs[0:BINS, 2] = cn

    sext = np.concatenate([s, [1.0]]).astype(np.float32)  # [101]

    in_maps = []
    for cid in range(NCORES):
        xc = x[cid * NTOK:(cid + 1) * NTOK]
        # permute: chunk column j*128+p <- chunk token 4p+j so that o_ps
        # partition p of subtile j holds token 4p+j -> contiguous store
        xp = xc.reshape(NCH, 128, NSUB).transpose(0, 2, 1).reshape(NTOK)
        xbs = np.zeros((128, NTOK), dtype=bf)
        xbs[0:K1] = (sext[:, None] * xp[None, :]).astype(bf)
        in_maps.append({"xbs": xbs, "wpack": wpack, "vecs": vecs})
    return in_maps


def _run(in_maps, trace=False, **kw):
    from concourse.bass_utils import run_bass_kernel_spmd
    if "nc" not in _CACHE:
        _CACHE["nc"] = _build_nc()
    return run_bass_kernel_spmd(_CACHE["nc"], in_maps,
                                list(range(NCORES)), trace=trace, **kw)


def kernel(**inputs):
    in_maps = _prep_in_maps(inputs["x"], inputs["w1"], inputs["b1"],
                            inputs["w2"], inputs["b2"], inputs["emb"])
    res = _run(in_maps)
    out = np.stack([res.results[c]["out"] for c in range(NCORES)])
    return out.reshape(B, S, DIM).astype(np.float32, copy=False)
